# revision 18
# baseline (speedup 1.0000x reference)
"""Trainium2 Bass kernel for nn_InvestigationBlock (dense transformer block).

Block: LN1 -> qkv -> polynomial-softmax attention -> proj -> +residual
       -> LN2 -> fc1 -> PolyGELU -> fc2 -> +residual

Sharding (8 cores, no collectives): core c handles batch b=c//2 and
query-token half s=c%2 (1024 of 2048 tokens). Each core computes k/v for
the full 2048 tokens of its batch element, everything else only for its
1024 query rows. Output rows are exact and disjoint; the host
concatenates.

Perf strategy (v2, fp8):
 - All GEMMs run in fp8(e4m3) with MatmulPerfMode.DoubleRow: each matmul
   instruction consumes TWO 128-deep contraction chunks ([P, 2, N] APs).
   Weights are pre-scaled by SW=32 on the host so their N(0,0.02) values
   land in e4m3's normal range; the 1/SW^k factors are folded into the
   ACT evacuation scale/bias constants (zero extra cost).
 - Attention scores also use DoubleRow via a zero-padded pair: qT8/kT8
   are stored [64, kc, 2, tok] with the second pair slot memset to 0, so
   S = k0.T@q0 + 0 at DoubleRow speed.
 - The poly+clamp (a*S^2+b*S+c, max eps) alternates between the Scalar
   engine (ACT Square with folded scale/bias) and the Vector engine
   (3-op Horner) per score tile to balance engine load; output is fp8
   `at` pair tiles [P, 2, 512] feeding the A@V DoubleRow matmul, with a
   ones-column on V so the row-sum rides along as PSUM row 64.
 - LN transposes to feature-major run on the PE (bf16 identity moving,
   1 cyc/row) instead of XBAR DMA (which serialized ~240us of kernel
   time); evacuation ACTs cast straight to fp8.
 - proj is computed token-major directly (attnT stationary, W moving)
   so its residual add needs no transpose; fc2 stays feature-major
   (deeper contraction) with bf16 PE transposes for the residual.
 - Residual stream stays fp32 token-major; LN gamma/beta folded into
   the following matmul's weights on the host.
"""

import os
import sys

for _p in ("/opt/trn_rl_repo", os.path.expanduser("~/.axon_site/_ro/trn_rl_repo")):
    if os.path.isdir(_p) and _p not in sys.path:
        sys.path.insert(0, _p)

import math
from contextlib import ExitStack

import ml_dtypes
import numpy as np

import concourse.bass as bass
import concourse.mybir as mybir
import concourse.tile as tile
from concourse import bacc
from concourse.bass_utils import run_bass_kernel_spmd
from concourse.masks import make_identity

F32 = mybir.dt.float32
BF16 = mybir.dt.bfloat16
F8 = mybir.dt.float8e4
DR = mybir.MatmulPerfMode.DoubleRow

DIM = 768
HEADS = 12
HD = 64
HIDDEN = 4 * DIM
NTOK = 2048
NQ = 1024
NB = 4
SCALE = HD ** -0.5
LN_EPS = 1e-5
P = 128

KC = DIM // P          # 6 contraction chunks for DIM
KP = KC // 2           # 3 DoubleRow pairs for DIM
TC_KV = NTOK // P      # 16 token tiles (kv)
TC_Q = NQ // P         # 8 token tiles (q)
QCH = NQ // 512        # 2 query chunks of 512
MC_H = HIDDEN // P     # 24 feature chunks of hidden
MP_H = MC_H // 2       # 12 DoubleRow pairs for hidden

SW = 32.0   # weight scale (fp8 range)
SA = 16.0   # attention poly output scale
SG = 16.0   # gelu output scale
SP = 128.0  # attnT (proj input) scale


def _f(x):
    return float(np.asarray(x))


class Cfg:
    """Host-folded constants baked into the program."""

    def __init__(self, inputs):
        a, b, c = _f(inputs["attn_a"]), _f(inputs["attn_b"]), _f(inputs["attn_c"])
        ga, gb, gc = _f(inputs["gelu_a"]), _f(inputs["gelu_b"]), _f(inputs["gelu_c"])
        assert a > 0 and ga > 0
        # a*(Sx)^2 + b*(Sx) + c = (sa*S*x + b/(2sa))^2 + (c - b^2/(4a))
        sa = math.sqrt(a)
        rsa = math.sqrt(SA)
        # scores psum holds SW^2 * (q.k); at' = SA * poly
        self.attn_scale = rsa * sa * SCALE / (SW * SW)
        self.attn_bias = rsa * b / (2 * sa)
        self.attn_d = SA * (c - b * b / (4 * a))
        self.attn_eps = SA * 1e-6
        # rr = (SW/SP)*(av64 + SA*1e-8); rb = 1/rr; attnT = av*rb = SP*attn
        self.rr_scale = SW / SP
        self.rr_bias = SW * SA * 1e-8 / SP
        sg = math.sqrt(ga)
        rsg = math.sqrt(SG)
        # fc1 runs in bf16 (unscaled): psum holds u. gT = SG*(gelu(u) - d);
        # the d*colsum(W2) correction is folded into the fc2 bias on host.
        self.gelu_scale = rsg * sg
        self.gelu_d_raw = gc - gb * gb / (4 * ga)
        self.proj_evac = 1.0 / (SP * SW)
        self.fc2_evac = 1.0 / SG


def build_nc(cfg, v_bias_nonzero, qk_bias_nonzero, pb_nonzero, f2b_nonzero):
    nc = bacc.Bacc(None, target_bir_lowering=False)

    x_kv = nc.dram_tensor("x_kv", [NTOK, DIM], F32, kind="ExternalInput").ap()
    x_q = nc.dram_tensor("x_q", [NQ, DIM], F32, kind="ExternalInput").ap()
    w_qkv = nc.dram_tensor("w_qkv", [DIM, 3 * DIM], F8, kind="ExternalInput").ap()
    w_proj = nc.dram_tensor("w_proj", [DIM, DIM], F8, kind="ExternalInput").ap()
    w_fc1 = nc.dram_tensor("w_fc1", [DIM, HIDDEN], BF16, kind="ExternalInput").ap()
    w_fc2 = nc.dram_tensor("w_fc2", [HIDDEN, DIM], BF16, kind="ExternalInput").ap()
    # per-out-feature bias vectors (fp32), stored as [chunks, 128]
    b_qk = nc.dram_tensor("b_qk", [2 * KC, P], F32, kind="ExternalInput").ap()
    b_v = nc.dram_tensor("b_v", [DIM], F32, kind="ExternalInput").ap()
    b_proj = nc.dram_tensor("b_proj", [DIM], F32, kind="ExternalInput").ap()
    b_fc2 = nc.dram_tensor("b_fc2", [KC, P], F32, kind="ExternalInput").ap()
    b_gelu = nc.dram_tensor("b_gelu", [MC_H, P], F32, kind="ExternalInput").ap()
    y = nc.dram_tensor("y", [NQ, DIM], F32, kind="ExternalOutput").ap()

    with tile.TileContext(nc) as tc, ExitStack() as ctx:
        singles = ctx.enter_context(tc.tile_pool(name="singles", bufs=1))

        ident_bf = singles.tile([P, P], BF16)
        make_identity(nc, ident_bf)

        eps_sb = singles.tile([P, 1], F32)
        nc.vector.memset(eps_sb, LN_EPS)
        ab_sb = singles.tile([P, 1], F32)
        nc.vector.memset(ab_sb, cfg.attn_bias)
        rrb_sb = singles.tile([P, 1], F32)
        nc.vector.memset(rrb_sb, cfg.rr_bias)

        b_qk_sb = singles.tile([P, 2 * KC], F32)
        nc.sync.dma_start(b_qk_sb, b_qk.rearrange("c p -> p c"))
        b_gelu_sb = singles.tile([P, MC_H], F32)
        nc.sync.dma_start(b_gelu_sb, b_gelu.rearrange("c p -> p c"))
        b_fc2_sb = singles.tile([P, KC], F32)
        nc.sync.dma_start(b_fc2_sb, b_fc2.rearrange("c p -> p c"))
        if v_bias_nonzero:
            bv_row = singles.tile([1, DIM], F32)
            nc.sync.dma_start(bv_row, b_v[None, :])
            bv_b = singles.tile([P, DIM], F32)
            nc.gpsimd.partition_broadcast(bv_b, bv_row)
        if pb_nonzero:
            bp_row = singles.tile([1, DIM], F32)
            nc.sync.dma_start(bp_row, b_proj[None, :])
            bp_b = singles.tile([P, DIM], F32)
            nc.gpsimd.partition_broadcast(bp_b, bp_row)
        # residual stream tiles (fp32 token-major); x2 overwrites xq in place
        xq_tiles = [singles.tile([P, DIM], F32, name=f"xq{t}") for t in range(TC_Q)]
        x2_tiles = xq_tiles

        # pool B: h2T lives from proj/LN2 phase to the end
        poolB = ctx.enter_context(tc.tile_pool(name="poolB", bufs=1))
        h2T = poolB.tile([P, KC, NQ], BF16, name="h2T")

        # pool A2: lives through attention + proj
        ctxA2 = ExitStack()
        poolA2 = ctxA2.enter_context(tc.tile_pool(name="poolA2", bufs=1))
        # q/k feature-major fp8 with a zeroed DoubleRow pair slot:
        # [dpart(128=2 heads x 64), kc, pair, tok]
        qT = poolA2.tile([P, KC, 2, NQ], F8, name="qT")
        nc.vector.memset(qT[:, :, 1, :], 0.0)
        kT = poolA2.tile([P, KC, 2, NTOK], F8, name="kT")
        nc.vector.memset(kT[:, :, 1, :], 0.0)
        # v token-major with per-head ones column, padded to 80 so the
        # DoubleRow pair stride (kt dim) is 16B-aligned:
        # [ktok, head, kt, 64+1(+pad)]
        v_sb = poolA2.tile([P, HEADS, TC_KV, 80], F8, name="v_sb")
        nc.vector.memset(v_sb[:, :, :, HD:HD + 1], 1.0)
        attnT = poolA2.tile([P, KC, NQ], F8, name="attnT")
        wproj_sb = poolA2.tile([P, KC, DIM], F8, name="wproj_sb")
        nc.sync.dma_start(wproj_sb, w_proj.rearrange("(c p) o -> p c o", p=P))

        # pool A1: LN1 + qkv only
        ctxA1 = ExitStack()
        poolA1 = ctxA1.enter_context(tc.tile_pool(name="poolA1", bufs=1))
        wqkv_sb = poolA1.tile([P, KC, 3 * DIM], F8, name="wqkv_sb")
        nc.sync.dma_start(wqkv_sb, w_qkv.rearrange("(c p) o -> p c o", p=P))
        hkvT = poolA1.tile([P, KC, NTOK], F8, name="hkvT")
        hqT = poolA1.tile([P, KC, NQ], F8, name="hqT")

        def ln_tile(pool, src_tile, out_bf):
            """token-major LN: out_bf = (x - mean(x)) * rsqrt(var(x)+eps)."""
            stats = pool.tile([P, 3, 6], F32, tag="stats", name="stats")
            for sg in range(3):
                nc.vector.bn_stats(stats[:, sg], src_tile[:, sg * 256:(sg + 1) * 256])
            mv = pool.tile([P, 2], F32, tag="mv", name="mv")
            nc.vector.bn_aggr(mv, stats)
            rstd = pool.tile([P, 1], F32, tag="rstd", name="rstd")
            nc.scalar.activation(rstd, mv[:, 1:2],
                                 mybir.ActivationFunctionType.Sqrt, bias=eps_sb)
            nc.vector.reciprocal(rstd, rstd)
            nc.vector.tensor_scalar(out_bf, src_tile, mv[:, 0:1], rstd,
                                    mybir.AluOpType.subtract, mybir.AluOpType.mult)

        def transpose_group(ps_pool, hts, dstT, t0):
            """PE-transpose a group of token-major bf16 tiles [P, DIM] into
            feature-major dstT[:, fc, t0*P : (t0+G)*P] with one evacuation
            ACT per feature chunk."""
            G = len(hts)
            for fc in range(KC):
                tp = ps_pool.tile([P, 4, P], BF16, tag="tr", name="tr")
                for j, ht in enumerate(hts):
                    nc.tensor.transpose(tp[:, j], ht[:, fc * P:(fc + 1) * P],
                                        ident_bf)
                nc.scalar.activation(dstT[:, fc, t0 * P:(t0 + G) * P],
                                     tp[:, :G].rearrange("p a b -> p (a b)"),
                                     mybir.ActivationFunctionType.Copy)

        def evac(dst, src, bias_ap=None, scale=None):
            if bias_ap is None and scale is None:
                nc.scalar.activation(dst, src, mybir.ActivationFunctionType.Copy)
            else:
                kw = {}
                if bias_ap is not None:
                    kw["bias"] = bias_ap
                if scale is not None:
                    kw["scale"] = scale
                nc.scalar.activation(dst, src,
                                     mybir.ActivationFunctionType.Identity, **kw)

        # ---------------- LN1 + transpose + v/k/q GEMMs ----------------
        with tc.tile_pool(name="ln", bufs=3) as ln_pool, \
             tc.tile_pool(name="tr_ps", bufs=3, space="PSUM") as tr_ps, \
             tc.tile_pool(name="qkv_ps", bufs=3, space="PSUM") as qkv_ps:

            def v_gemm(t):
                # v token-major: stationary h^T chunk (M=128 tok), moving W
                for half in range(2):  # heads 0..7 then 8..11
                    ncol = 512 if half == 0 else 256
                    nh = ncol // HD
                    pt = qkv_ps.tile([P, 512], F32, tag="mm", name="pt")[:, :ncol]
                    for kp in range(KP):
                        nc.tensor.matmul(
                            pt,
                            hkvT[:, 2 * kp:2 * kp + 2, t * P:(t + 1) * P],
                            wqkv_sb[:, 2 * kp:2 * kp + 2,
                                    2 * DIM + half * 512:2 * DIM + half * 512 + ncol],
                            start=(kp == 0), stop=(kp == KP - 1), perf_mode=DR)
                    h0 = half * 8
                    dst = v_sb[:, h0:h0 + nh, t, 0:HD]
                    src = pt.rearrange("p (h d) -> p h d", d=HD)
                    if v_bias_nonzero:
                        nc.vector.tensor_tensor(
                            dst, src,
                            bv_b[:, half * 512:half * 512 + ncol]
                            .rearrange("p (h d) -> p h d", d=HD),
                            mybir.AluOpType.add)
                    else:
                        nc.scalar.activation(dst, src,
                                             mybir.ActivationFunctionType.Copy)

            def qk_gemm(dst, rhs, qc, off):
                # feature-major q^T/k^T chunk qc (512 cols)
                for mc in range(KC):
                    pt = qkv_ps.tile([P, 512], F32, tag="mm", name="mm")
                    for kp in range(KP):
                        nc.tensor.matmul(
                            pt,
                            wqkv_sb[:, 2 * kp:2 * kp + 2,
                                    off + mc * P:off + (mc + 1) * P],
                            rhs[:, 2 * kp:2 * kp + 2, qc * 512:(qc + 1) * 512],
                            start=(kp == 0), stop=(kp == KP - 1), perf_mode=DR)
                    bias_ap = None
                    if qk_bias_nonzero:
                        i = (off // DIM) * KC + mc
                        bias_ap = b_qk_sb[:, i:i + 1]
                    evac(dst[:, mc, 0, qc * 512:(qc + 1) * 512], pt, bias_ap)

            for tg in range(TC_KV // 4):
                hts = []
                for j in range(4):
                    t = 4 * tg + j
                    xt = ln_pool.tile([P, DIM], F32, tag="xt", name="xt")
                    nc.sync.dma_start(xt, x_kv[t * P:(t + 1) * P, :])
                    ht = ln_pool.tile([P, DIM], BF16, tag=f"ht{j}", name="ht")
                    ln_tile(ln_pool, xt, ht)
                    hts.append(ht)
                transpose_group(tr_ps, hts, hkvT, 4 * tg)
                for j in range(4):
                    v_gemm(4 * tg + j)
                qk_gemm(kT, hkvT, tg, DIM)
            for tg in range(TC_Q // 4):
                hts = []
                for j in range(4):
                    t = 4 * tg + j
                    nc.sync.dma_start(xq_tiles[t], x_q[t * P:(t + 1) * P, :])
                    ht = ln_pool.tile([P, DIM], BF16, tag=f"ht{j}", name="ht")
                    ln_tile(ln_pool, xq_tiles[t], ht)
                    hts.append(ht)
                transpose_group(tr_ps, hts, hqT, 4 * tg)
                qk_gemm(qT, hqT, tg, 0)

        ctxA1.close()

        # ---------------- attention ----------------
        NPAIR = TC_KV // 2
        with tc.tile_pool(name="at", bufs=3) as at_pool, \
             tc.tile_pool(name="sc_ps", bufs=3, space="PSUM") as sc_ps, \
             tc.tile_pool(name="av_ps", bufs=2, space="PSUM") as av_ps:
            for h in range(HEADS):
                base = (h % 2) * HD
                g = h // 2
                for qc in range(QCH):
                    av = av_ps.tile([HD + 1, 512], F32, tag="av", name="av")
                    for p_ in range(NPAIR):
                        atp = at_pool.tile([P, 2, 512], F8, tag="atp", name="atp")
                        stp = sc_ps.tile([P, 2, 512], F32, tag="sc", name="sc")
                        for i in range(2):
                            kt = 2 * p_ + i
                            nc.tensor.matmul(
                                stp[:, i],
                                kT[base:base + HD, g, :, kt * P:(kt + 1) * P],
                                qT[base:base + HD, g, :, qc * 512:(qc + 1) * 512],
                                start=True, stop=True, perf_mode=DR)
                        # poly: Square(scale*S + b) on Scalar over the pair,
                        # then (+d, max eps) -> fp8 on Vector over the pair
                        stg = at_pool.tile([P, 2, 512], BF16, tag="stg",
                                           name="stg")
                        nc.scalar.activation(
                            stg.rearrange("p a b -> p (a b)"),
                            stp.rearrange("p a b -> p (a b)"),
                            mybir.ActivationFunctionType.Square,
                            bias=ab_sb, scale=cfg.attn_scale)
                        nc.vector.tensor_scalar(
                            atp.rearrange("p a b -> p (a b)"),
                            stg.rearrange("p a b -> p (a b)"),
                            cfg.attn_d, cfg.attn_eps,
                            mybir.AluOpType.add, mybir.AluOpType.max)
                        nc.tensor.matmul(av,
                                         v_sb[:, h, 2 * p_:2 * p_ + 2, 0:HD + 1],
                                         atp,
                                         start=(p_ == 0), stop=(p_ == NPAIR - 1),
                                         perf_mode=DR)
                    # normalize: attnT[d, q] = av[d, q] * SP/(SW*(av64 + SA*eps))
                    rr = at_pool.tile([1, 512], F32, tag="rr", name="rr")
                    nc.scalar.activation(rr, av[HD:HD + 1, :],
                                         mybir.ActivationFunctionType.Identity,
                                         bias=rrb_sb[0:1, :], scale=cfg.rr_scale)
                    nc.vector.reciprocal_approx_fast(rr, rr)
                    rb = at_pool.tile([HD, 512], F32, tag="rb", name="rb")
                    nc.gpsimd.partition_broadcast(rb, rr)
                    nc.vector.tensor_tensor(
                        attnT[base:base + HD, g, qc * 512:(qc + 1) * 512],
                        av[0:HD, :], rb, mybir.AluOpType.mult)

        # ---------------- proj + residual -> x2 (token-major direct) -----
        # ---------------- then LN2 -> h2T ----------------
        with tc.tile_pool(name="pj", bufs=3) as pj_pool, \
             tc.tile_pool(name="pj_ps", bufs=4, space="PSUM") as pj_ps, \
             tc.tile_pool(name="pj_tr", bufs=3, space="PSUM") as pj_tr:
            for tg in range(TC_Q // 4):
                hts = []
                for j in range(4):
                    t = 4 * tg + j
                    for mc in range(KC):
                        pt = pj_ps.tile([P, P], F32, tag="mm", name="mm")
                        for kp in range(KP):
                            nc.tensor.matmul(
                                pt,
                                attnT[:, 2 * kp:2 * kp + 2, t * P:(t + 1) * P],
                                wproj_sb[:, 2 * kp:2 * kp + 2,
                                         mc * P:(mc + 1) * P],
                                start=(kp == 0), stop=(kp == KP - 1),
                                perf_mode=DR)
                        nc.vector.scalar_tensor_tensor(
                            x2_tiles[t][:, mc * P:(mc + 1) * P], pt,
                            cfg.proj_evac,
                            xq_tiles[t][:, mc * P:(mc + 1) * P],
                            mybir.AluOpType.mult, mybir.AluOpType.add)
                    if pb_nonzero:
                        nc.vector.tensor_tensor(x2_tiles[t], x2_tiles[t], bp_b,
                                                mybir.AluOpType.add)
                    ht = pj_pool.tile([P, DIM], BF16, tag=f"ht{j}", name="ht")
                    ln_tile(pj_pool, x2_tiles[t], ht)
                    hts.append(ht)
                transpose_group(pj_tr, hts, h2T, 4 * tg)

        ctxA2.close()  # release poolA2

        # ---------------- MLP + residual -> y ----------------
        with tc.tile_pool(name="mlp", bufs=2) as mlp_pool, \
             tc.tile_pool(name="mlp_ps", bufs=3, space="PSUM") as mlp_ps, \
             tc.tile_pool(name="mtr_ps", bufs=4, space="PSUM") as mtr_ps:
            wfc1_sb = mlp_pool.tile([P, KC, HIDDEN], BF16, tag="wfc1", bufs=1,
                                    name="wfc1")
            nc.sync.dma_start(wfc1_sb, w_fc1.rearrange("(c p) o -> p c o", p=P))
            wfc2_sb = mlp_pool.tile([P, MC_H, DIM], BF16, tag="wfc2", bufs=1,
                                    name="wfc2")
            nc.sync.dma_start(wfc2_sb, w_fc2.rearrange("(c p) o -> p c o", p=P))
            for qc in range(QCH):
                gT = mlp_pool.tile([P, MC_H, 512], BF16, tag="gT", bufs=2, name="gT")
                for mc in range(MC_H):
                    pt = mlp_ps.tile([P, 512], F32, tag="mm", name="mm")
                    for kc in range(KC):
                        nc.tensor.matmul(
                            pt, wfc1_sb[:, kc, mc * P:(mc + 1) * P],
                            h2T[:, kc, qc * 512:(qc + 1) * 512],
                            start=(kc == 0), stop=(kc == KC - 1))
                    # PolyGELU square part: SG*(sg*u + t)^2 -> fp8 directly;
                    # the +SG*d constant is folded into the fc2 bias on host.
                    nc.scalar.activation(gT[:, mc], pt,
                                         mybir.ActivationFunctionType.Square,
                                         bias=b_gelu_sb[:, mc:mc + 1],
                                         scale=cfg.gelu_scale)
                f2T = mlp_pool.tile([P, KC, 512], BF16, tag="f2T", bufs=2,
                                    name="f2T")
                for mc in range(KC):
                    pt = mlp_ps.tile([P, 512], F32, tag="mm", name="mm")
                    for kc in range(MC_H):
                        nc.tensor.matmul(
                            pt, wfc2_sb[:, kc, mc * P:(mc + 1) * P],
                            gT[:, kc, :],
                            start=(kc == 0), stop=(kc == MC_H - 1))
                    evac(f2T[:, mc], pt, bias_ap=b_fc2_sb[:, mc:mc + 1],
                         scale=cfg.fc2_evac)
                for qt in range(4):
                    t = qc * 4 + qt
                    yt = mlp_pool.tile([P, DIM], F32, tag="yt", bufs=2, name="yt")
                    for mc in range(KC):
                        tp = mtr_ps.tile([P, P], BF16, tag="tr", name="tr")
                        nc.tensor.transpose(tp, f2T[:, mc, qt * P:(qt + 1) * P],
                                            ident_bf)
                        nc.vector.scalar_tensor_tensor(
                            yt[:, mc * P:(mc + 1) * P], tp, 1.0,
                            x2_tiles[t][:, mc * P:(mc + 1) * P],
                            mybir.AluOpType.mult, mybir.AluOpType.add)
                    nc.sync.dma_start(y[t * P:(t + 1) * P, :], yt)

    nc.compile()
    return nc


_CACHED = {}


def _prep(inputs):
    """Host-side folding: returns (cfg, cache key, build flags, in_maps)."""
    ins = {k: np.asarray(v) for k, v in inputs.items()}
    x = ins["x"].astype(np.float32)
    cfg = Cfg(ins)

    ln1_g, ln1_b = ins["ln1_g"].astype(np.float32), ins["ln1_b"].astype(np.float32)
    ln2_g, ln2_b = ins["ln2_g"].astype(np.float32), ins["ln2_b"].astype(np.float32)
    qkv_w = ins["qkv_w"].astype(np.float32)
    fc1_w = ins["fc1_w"].astype(np.float32)

    qkv_w_eff = ln1_g[:, None] * qkv_w
    qkv_b_eff = ins["qkv_b"].astype(np.float32) + ln1_b @ qkv_w
    fc1_w_eff = ln2_g[:, None] * fc1_w
    fc1_b_eff = ins["fc1_b"].astype(np.float32) + ln2_b @ fc1_w

    b_qk = SW * qkv_b_eff[:2 * DIM]
    b_v = SW * qkv_b_eff[2 * DIM:]
    b_proj = ins["proj_b"].astype(np.float32)
    b_fc2 = ins["fc2_b"].astype(np.float32)
    # fc1 bias folded into the gelu ACT bias vector:
    # Square((rsg*sg/SW)*u' + rsg*(sg*b1 + gb/(2sg))) + SG*d
    rsg = math.sqrt(SG)
    sg_ = math.sqrt(_f(ins["gelu_a"]))
    b_gelu = rsg * (sg_ * fc1_b_eff + _f(ins["gelu_b"]) / (2 * sg_))

    qk_bias_nonzero = bool(np.any(b_qk != 0.0))
    v_bias_nonzero = bool(np.any(b_v != 0.0))
    pb_nonzero = bool(np.any(b_proj != 0.0))
    f2b_nonzero = bool(np.any(b_fc2 != 0.0))

    key = (qk_bias_nonzero, v_bias_nonzero, pb_nonzero, f2b_nonzero,
           cfg.attn_scale, cfg.attn_bias, cfg.attn_d,
           cfg.gelu_scale, cfg.gelu_d_raw)
    flags = (v_bias_nonzero, qk_bias_nonzero, pb_nonzero, f2b_nonzero)

    f8 = ml_dtypes.float8_e4m3
    bf = ml_dtypes.bfloat16
    fc2_w = ins["fc2_w"].astype(np.float32)
    # gT holds SG*(gelu(u) - d); compensate d through fc2's column sums
    gd = _f(ins["gelu_c"]) - _f(ins["gelu_b"]) ** 2 / (4 * _f(ins["gelu_a"]))
    b_fc2_eff = b_fc2 + gd * fc2_w.sum(axis=0)
    common = {
        "w_qkv": np.ascontiguousarray((SW * qkv_w_eff).astype(f8)),
        "w_proj": np.ascontiguousarray(
            (SW * ins["proj_w"].astype(np.float32)).astype(f8)),
        "w_fc1": np.ascontiguousarray(fc1_w_eff.astype(bf)),
        "w_fc2": np.ascontiguousarray(fc2_w.astype(bf)),
        "b_qk": np.ascontiguousarray(b_qk.reshape(2 * KC, P)),
        "b_v": np.ascontiguousarray(b_v),
        "b_proj": np.ascontiguousarray(b_proj),
        "b_fc2": np.ascontiguousarray(b_fc2_eff.reshape(KC, P)),
        "b_gelu": np.ascontiguousarray(b_gelu.reshape(MC_H, P)),
    }
    in_maps = []
    for c in range(8):
        b, s = c // 2, c % 2
        m = dict(common)
        m["x_kv"] = np.ascontiguousarray(x[b])
        m["x_q"] = np.ascontiguousarray(x[b, s * NQ:(s + 1) * NQ])
        in_maps.append(m)
    return cfg, key, flags, in_maps


def make_in_maps(inputs):
    return _prep(inputs)[3]


def kernel(**inputs) -> np.ndarray:
    cfg, key, flags, in_maps = _prep(inputs)
    if key not in _CACHED:
        _CACHED[key] = build_nc(cfg, *flags)
    nc = _CACHED[key]

    res = run_bass_kernel_spmd(nc, in_maps, core_ids=list(range(8)))

    out = np.empty((NB, NTOK, DIM), dtype=np.float32)
    for c in range(8):
        b, s = c // 2, c % 2
        out[b, s * NQ:(s + 1) * NQ] = res.results[c]["y"]
    return out
